# revision 16
# baseline (speedup 1.0000x reference)
"""Trainium2 Bass kernel for a 2-layer GCN + MLP heads (CNE-style model).

Math (eval mode, matches the PyG GCNConv reference):
    src,dst = edges + self-loops;  deg = indegree(dst);  dinv = deg^-0.5
    norm factorizes: msg_e = dinv[src]*h[src],  out[d] = dinv[d]*sum_{e->d} msg_e
    xZ1 = relu(gcn(x @ W1) + b1); xZ2 = gcn(xZ1 @ W2) + b2
    heads: s1/s0 = lrelu(xZ2 @ Wy1/Wy0 + b), tprob = lrelu(lrelu(xZ2@Wp1+bp1)@Wp2+bp2)
    y1 = s1[treat], yc0 = s0[treat], y0 = s0[control], yc1 = s1[control]

Distribution: nodes partitioned into 8 contiguous dst-blocks (one per core).
Each core redundantly computes the full layer-1 feature table (cheap GEMM),
aggregates only its own edges (gather via SWDGE dma_gather of 256-byte rows),
then an on-chip AllGather exchanges layer-1 outputs before layer 2.

Numerics: gather tables are stored as bf16 hi/lo split pairs ([hi|lo] in one
256B row); segment-reduction is one bf16 matmul per 128-edge tile against an
on-chip-built one-hot indicator; hi+lo partial sums are combined per window.
This keeps ~fp16-accuracy end-to-end while using 1-cycle/row bf16 matmuls.
"""
import sys
import numpy as np

for _p in ("/opt/trn_rl_repo",):
    if _p not in sys.path:
        sys.path.insert(0, _p)

import ml_dtypes
import concourse.bass as bass
import concourse.tile as tile
from concourse import bacc, mybir, library_config
from concourse.bass_utils import run_bass_kernel_spmd

F32 = mybir.dt.float32
BF16 = mybir.dt.bfloat16
I16 = mybir.dt.int16
AF = mybir.ActivationFunctionType
ALU = mybir.AluOpType

P = 128          # partitions / window width / edge-tile size
CHUNK = 4096     # gather idxs per dma_gather call (32 tiles)
TPC = CHUNK // P  # tiles per gather chunk
IND_B = 16       # tiles per indicator build op
DR_CH = 512      # dst_rel cols per DMA chunk


def _ceil(a, b):
    return -(-a // b)


class Cfg:
    def __init__(self, n, in_dim, h, ncores, nblocks):
        self.N = n
        self.IN = in_dim
        self.H = h
        self.NC = ncores
        self.NB = nblocks
        assert n % ncores == 0 and n % nblocks == 0
        self.BLK = n // ncores       # dst nodes per core
        self.SRCB = n // nblocks     # src rows per gather block
        assert self.SRCB <= 32767, "gather idx must fit int16"
        self.NW = _ceil(self.BLK, P)          # windows per core
        self.WPAD = self.NW * P               # padded block width
        self.TBL = _ceil(n, 4 * P) * 4 * P    # padded table rows (mult of 512)


def preprocess(cfg, x, edge_index):
    """Host-side graph partitioning + schedule construction (shared across cores)."""
    n = cfg.N
    src = np.concatenate([edge_index[0], np.arange(n, dtype=np.int64)])
    dst = np.concatenate([edge_index[1], np.arange(n, dtype=np.int64)])
    deg = np.bincount(dst, minlength=n).astype(np.float32)
    dinv = deg ** -0.5

    core = dst // cfg.BLK
    win = (dst % cfg.BLK) // P
    blk = src // cfg.SRCB
    grp = (win * cfg.NB + blk).astype(np.int64)

    # per-core sorted edge groups
    per_core = []
    ngrp = cfg.NW * cfg.NB
    counts = np.zeros((cfg.NC, ngrp), np.int64)
    for c in range(cfg.NC):
        m = core == c
        g = grp[m]
        order = np.argsort(g, kind="stable")
        per_core.append((src[m][order], dst[m][order], g[order]))
        counts[c] = np.bincount(g, minlength=ngrp)

    tiles = _ceil(counts, P)                 # [NC, ngrp]
    T = tiles.max(axis=0)                    # shared schedule [ngrp]
    ntile = int(T.sum())
    ntile_pad = _ceil(ntile, DR_CH) * DR_CH

    # stream slot layout
    slots = T * P                            # [ngrp] slots per group
    # per-block stream lengths
    Lb = np.zeros(cfg.NB, np.int64)
    for b in range(cfg.NB):
        Lb[b] = slots[b::cfg.NB].sum()
    Lb_pad = (_ceil(Lb, CHUNK) * CHUNK).astype(np.int64)

    # build per-core idx streams + dst_rel
    idx_streams = []   # [NC][NB] -> [128, Lb_pad/16] int16
    dstrel = np.full((cfg.NC, ntile_pad, P), -1.0, np.float32)
    for c in range(cfg.NC):
        s_arr, d_arr, g_arr = per_core[c]
        gstart = np.concatenate([[0], np.cumsum(counts[c])])
        streams = [np.zeros(Lb_pad[b], np.int16) for b in range(cfg.NB)]
        scur = np.zeros(cfg.NB, np.int64)
        tcur = 0
        for w in range(cfg.NW):
            for b in range(cfg.NB):
                gi = w * cfg.NB + b
                cnt = counts[c][gi]
                e0 = gstart[gi]
                tgt = T[gi] * P
                sl = streams[b]
                sl[scur[b]:scur[b] + cnt] = (s_arr[e0:e0 + cnt] - b * cfg.SRCB).astype(np.int16)
                # dst_rel for this group's tiles
                dr = np.full(tgt, -1.0, np.float32)
                dr[:cnt] = (d_arr[e0:e0 + cnt] - (c * cfg.BLK + w * P)).astype(np.float32)
                dstrel[c, tcur:tcur + T[gi]] = dr.reshape(T[gi], P)
                scur[b] += tgt
                tcur += T[gi]
        assert tcur == ntile
        idx_streams.append([
            np.tile(streams[b].reshape(-1, 16).T, (8, 1)).copy() for b in range(cfg.NB)
        ])
    # dst_rel packed [NC, 128, ntile_pad]
    dstrel_packed = np.ascontiguousarray(dstrel.transpose(0, 2, 1))

    sched = dict(T=T, ntile=ntile, ntile_pad=ntile_pad, Lb_pad=Lb_pad)
    return sched, idx_streams, dstrel_packed, dinv


def build_program(cfg, sched):
    """Build the SPMD Bass program (same schedule for all cores)."""
    T = sched["T"]
    ntile_pad = sched["ntile_pad"]
    Lb_pad = sched["Lb_pad"]
    NB, NW, H, IN = cfg.NB, cfg.NW, cfg.H, cfg.IN
    H2 = 2 * H

    nc0 = bacc.Bacc("TRN2")
    dt = nc0.dram_tensor
    xt_d = dt("xt", [IN, cfg.TBL], F32, kind="ExternalInput")
    ix_d = [dt(f"ix{b}", [P, int(Lb_pad[b]) // 16], I16, kind="ExternalInput")
            for b in range(NB)]
    dr_d = dt("dr", [P, ntile_pad], F32, kind="ExternalInput")
    iota_d = dt("iota", [P, P], BF16, kind="ExternalInput")
    dinvr_d = dt("dinvr", [H, cfg.WPAD], F32, kind="ExternalInput")
    dinvc_d = dt("dinvc", [P, NW], F32, kind="ExternalInput")
    w1_d = dt("w1", [IN, H], F32, kind="ExternalInput")
    w2_d = dt("w2", [H, H], F32, kind="ExternalInput")
    wp1_d = dt("wp1", [H, H], F32, kind="ExternalInput")
    wp2_d = dt("wp2", [H, 2], F32, kind="ExternalInput")
    wy_d = dt("wy", [H, 2], F32, kind="ExternalInput")
    b1_d = dt("b1", [H, 1], F32, kind="ExternalInput")
    b2_d = dt("b2", [H, 1], F32, kind="ExternalInput")
    bp1_d = dt("bp1", [H, 1], F32, kind="ExternalInput")
    bp2_d = dt("bp2", [2, 1], F32, kind="ExternalInput")
    by_d = dt("by", [2, 1], F32, kind="ExternalInput")
    xz2_d = dt("xz2t", [H, cfg.WPAD], F32, kind="ExternalOutput")
    heads_d = dt("headst", [4, cfg.WPAD], F32, kind="ExternalOutput")

    import os
    PH = os.environ.get("K_PHASES", "ABCD")
    with tile.TileContext(nc0) as tc:
        nc = tc.nc
        nc.gpsimd.load_library(library_config.mlp)
        import contextlib
        ctx = contextlib.ExitStack()
        with ctx:
            cpool = ctx.enter_context(tc.tile_pool(name="consts", bufs=1))
            dram = ctx.enter_context(tc.tile_pool(name="dram", bufs=1, space="DRAM"))
            xtp = ctx.enter_context(tc.tile_pool(name="xt", bufs=3))
            tblp = ctx.enter_context(tc.tile_pool(name="tblpair", bufs=3))
            # PSUM budget: 8 banks total -> 2 (table) + 3 (windows) + 3 (mlp)
            ps_tbl = ctx.enter_context(tc.tile_pool(name="ps_tbl", bufs=2, space="PSUM"))
            ps_win = ctx.enter_context(tc.tile_pool(name="ps_win", bufs=3, space="PSUM"))
            ps_mlp = ctx.enter_context(tc.tile_pool(name="ps_mlp", bufs=3, space="PSUM"))
            gpools = [ctx.enter_context(tc.tile_pool(name=f"g{b}", bufs=2))
                      for b in range(NB)]
            ixpools = [ctx.enter_context(tc.tile_pool(name=f"ix{b}", bufs=2))
                       for b in range(NB)]
            indp = ctx.enter_context(tc.tile_pool(name="ind", bufs=2))
            drp = ctx.enter_context(tc.tile_pool(name="drel", bufs=2))
            winp = ctx.enter_context(tc.tile_pool(name="win", bufs=3))

            # ---- consts ----
            def cload(dram_t, shape, dtype=F32):
                t = cpool.tile(shape, dtype, tag=dram_t.name + "_c")
                nc.sync.dma_start(t[:], dram_t[:])
                return t

            iota_t = cload(iota_d, [P, P], BF16)
            w1_t = cload(w1_d, [IN, H])
            w2_t = cload(w2_d, [H, H])
            wp1_t = cload(wp1_d, [H, H])
            wp2_t = cload(wp2_d, [H, 2])
            wy_t = cload(wy_d, [H, 2])
            b1_t = cload(b1_d, [H, 1])
            b2_t = cload(b2_d, [H, 1])
            bp1_t = cload(bp1_d, [H, 1])
            bp2_t = cload(bp2_d, [2, 1])
            by_t = cload(by_d, [2, 1])
            dinvr_t = cpool.tile([H, cfg.WPAD], F32, tag="dinvr")
            nc.sync.dma_start(dinvr_t[:], dinvr_d[:])
            dinvc_t = cpool.tile([P, NW], F32, tag="dinvc")
            nc.sync.dma_start(dinvc_t[:], dinvc_d[:])

            # internal DRAM: tables + exchange buffers
            tbl1 = dram.tile([cfg.TBL, H2], BF16)
            h2loc = dram.tile([cfg.BLK, H2], BF16)
            tbl2 = dram.tile([cfg.N, H2], BF16)

            # ---- phase A: layer-1 table build (all nodes, redundant per core)
            nchunk = cfg.TBL // 512 if "A" in PH else 0
            for g in range(nchunk):
                xt_t = xtp.tile([IN, 512], F32, tag="xt")
                nc.sync.dma_start(xt_t[:], xt_d[:, g * 512:(g + 1) * 512])
                ps4 = ps_tbl.tile([P, 4, H], F32, space="PSUM", tag="ps4")
                for j in range(4):
                    nc.tensor.matmul(ps4[:, j, :], lhsT=xt_t[:, j * P:(j + 1) * P],
                                     rhs=w1_t[:], start=True, stop=True)
                pair = tblp.tile([P, 4, H2], BF16, tag="pair")
                nc.scalar.activation(pair[:, :, 0:H], ps4[:], AF.Copy)
                nc.vector.tensor_tensor(out=pair[:, :, H:H2], in0=ps4[:],
                                        in1=pair[:, :, 0:H], op=ALU.subtract)
                dview = tbl1[:].rearrange("(f p) d -> p f d", p=P)
                nc.sync.dma_start(dview[:, g * 4:(g + 1) * 4, :], pair[:])

            LITE = int(os.environ.get("K_LITE", "4"))
            # ---- aggregation pass (used for both layers) ----
            def agg_pass(table_ap, close_fn):
                scur = [0] * NB      # per-stream tile cursor
                gbufs = [None] * NB
                ixbufs = [None] * NB
                gchunk = [-1] * NB
                ind_t = None
                dr_t = None
                gcur = 0
                for w in range(NW):
                    pw = (ps_win.tile([P, P], F32, space="PSUM", tag="pw",
                                       name="pw")
                          if LITE >= 3 else None)
                    tot = sum(int(T[w * NB + b]) for b in range(NB))
                    done = 0
                    for b in range(NB):
                        for t in range(int(T[w * NB + b])):
                            s = scur[b]
                            scur[b] += 1
                            ck, slot = divmod(s, TPC)
                            if ck != gchunk[b]:
                                gchunk[b] = ck
                                ixbufs[b] = ixpools[b].tile([P, CHUNK // 16], I16,
                                                            tag=f"ixb{b}", name=f"ixb{b}")
                                nc.sync.dma_start(
                                    ixbufs[b][:],
                                    ix_d[b][:, ck * (CHUNK // 16):(ck + 1) * (CHUNK // 16)])
                                gbufs[b] = gpools[b].tile([P, TPC, H2], BF16,
                                                          tag=f"gb{b}", name=f"gb{b}")
                                nc.gpsimd.dma_gather(
                                    gbufs[b][:], table_ap[b * cfg.SRCB:(b + 1) * cfg.SRCB, :],
                                    ixbufs[b][:], CHUNK, CHUNK, H2,
                                    single_packet=False)
                            if LITE >= 2 and gcur % IND_B == 0:
                                if gcur % DR_CH == 0:
                                    dr_t = drp.tile([P, DR_CH], F32, tag="drt")
                                    nc.sync.dma_start(
                                        dr_t[:], dr_d[:, gcur:gcur + DR_CH])
                                ind_t = indp.tile([P, IND_B, P], BF16, tag="indt")
                                j = gcur % DR_CH
                                drs = dr_t[:, j:j + IND_B]
                                in0 = bass.AP(drs.tensor, drs.offset,
                                              list(drs.ap) + [[0, P]])
                                ios = iota_t[:]
                                in1 = bass.AP(ios.tensor, ios.offset,
                                              [ios.ap[0], [0, IND_B], ios.ap[1]])
                                nc.vector.tensor_tensor(out=ind_t[:], in0=in0,
                                                        in1=in1, op=ALU.is_equal)
                            if LITE >= 3:
                                nc.tensor.matmul(pw[:], lhsT=gbufs[b][:, slot, :],
                                                 rhs=ind_t[:, gcur % IND_B, :],
                                                 start=(done == 0), stop=(done == tot - 1))
                            done += 1
                            gcur += 1
                    if LITE >= 4:
                        close_fn(w, pw)

            # ---- phase B: layer 1 ----
            def close1(w, pw):
                lo = winp.tile([H, P], F32, tag="lo")
                nc.scalar.activation(lo[:], pw[H:P, :], AF.Copy)
                aggc = winp.tile([H, P], F32, tag="aggc")
                nc.vector.tensor_tensor(out=aggc[:], in0=lo[:], in1=pw[0:H, :],
                                        op=ALU.add)
                sc = winp.tile([H, P], F32, tag="sc")
                nc.vector.tensor_tensor(out=sc[:], in0=aggc[:],
                                        in1=dinvr_t[:, w * P:(w + 1) * P],
                                        op=ALU.mult)
                xz1 = winp.tile([H, P], F32, tag="xz1")
                nc.scalar.activation(xz1[:], sc[:], AF.Relu, bias=b1_t[:, 0:1])
                # fold W2: h2_w = dinv * (xz1_w @ W2), emitted as bf16 hi/lo pair
                p2 = ps_mlp.tile([P, H], F32, space="PSUM", tag="mlp")
                nc.tensor.matmul(p2[:], lhsT=xz1[:], rhs=w2_t[:], start=True, stop=True)
                t2 = winp.tile([P, H], F32, tag="t2")
                nc.vector.tensor_scalar_mul(t2[:], p2[:], dinvc_t[:, w:w + 1])
                h2r = winp.tile([P, H2], BF16, tag="h2r")
                nc.scalar.activation(h2r[:, 0:H], t2[:], AF.Copy)
                nc.vector.tensor_tensor(out=h2r[:, H:H2], in0=t2[:],
                                        in1=h2r[:, 0:H], op=ALU.subtract)
                rows = min(P, cfg.BLK - w * P)
                nc.sync.dma_start(h2loc[w * P:w * P + rows, :], h2r[0:rows, :])

            if "B" in PH:
                agg_pass(tbl1, close1)
            else:
                # keep tbl1 alive: dump a bf16 slice into xz2 output (bitcast)
                dbg = winp.tile([P, 64], BF16, tag="dbg")
                nc.sync.dma_start(dbg[:], tbl1[0:P, 0:64])
                nc.sync.dma_start(
                    xz2_d[0:H, 0:32].bitcast(BF16), dbg[0:H, :])

            # ---- phase C: exchange layer-1 outputs ----
            if ("C" not in PH) or os.environ.get("K_NO_COLLECTIVE"):
                nc.sync.dma_start(tbl2[0:cfg.BLK, :], h2loc[:])
            else:
                nc.gpsimd.collective_compute(
                    "AllGather", ALU.bypass,
                    replica_groups=[list(range(cfg.NC))],
                    ins=[h2loc.opt()], outs=[tbl2.opt()])

            # ---- phase D: layer 2 + heads ----
            def close2(w, pw):
                lo = winp.tile([H, P], F32, tag="lo")
                nc.scalar.activation(lo[:], pw[H:P, :], AF.Copy)
                aggc = winp.tile([H, P], F32, tag="aggc")
                nc.vector.tensor_tensor(out=aggc[:], in0=lo[:], in1=pw[0:H, :],
                                        op=ALU.add)
                sc = winp.tile([H, P], F32, tag="sc")
                nc.vector.tensor_tensor(out=sc[:], in0=aggc[:],
                                        in1=dinvr_t[:, w * P:(w + 1) * P],
                                        op=ALU.mult)
                xz2 = winp.tile([H, P], F32, tag="xz2")
                nc.vector.tensor_scalar_add(xz2[:], sc[:], b2_t[:, 0:1])
                nc.sync.dma_start(xz2_d[:, w * P:(w + 1) * P], xz2[:])
                # heads
                pp1 = ps_mlp.tile([H, P], F32, space="PSUM", tag="mlp")
                nc.tensor.matmul(pp1[:], lhsT=wp1_t[:], rhs=xz2[:], start=True, stop=True)
                p1 = winp.tile([H, P], F32, tag="p1")
                nc.vector.tensor_scalar_add(p1[:], pp1[:], bp1_t[:, 0:1])
                p1s = winp.tile([H, P], F32, tag="p1s")
                nc.scalar.activation(p1s[:], p1[:], AF.Copy, scale=0.01)
                nc.vector.tensor_tensor(out=p1[:], in0=p1[:], in1=p1s[:],
                                        op=ALU.max)
                hrow_p = winp.tile([2, P], F32, tag="hrowp")
                pp2 = ps_mlp.tile([2, P], F32, space="PSUM", tag="mlp")
                nc.tensor.matmul(pp2[:], lhsT=wp2_t[:], rhs=p1[:], start=True, stop=True)
                nc.vector.tensor_scalar_add(hrow_p[:], pp2[:], bp2_t[:, 0:1])
                hp_s = winp.tile([2, P], F32, tag="hps")
                nc.scalar.activation(hp_s[:], hrow_p[:], AF.Copy, scale=0.01)
                nc.vector.tensor_tensor(out=hrow_p[:], in0=hrow_p[:], in1=hp_s[:],
                                        op=ALU.max)
                hrow_s = winp.tile([2, P], F32, tag="hrows")
                pps = ps_mlp.tile([2, P], F32, space="PSUM", tag="mlp")
                nc.tensor.matmul(pps[:], lhsT=wy_t[:], rhs=xz2[:], start=True, stop=True)
                nc.vector.tensor_scalar_add(hrow_s[:], pps[:], by_t[:, 0:1])
                hs_s = winp.tile([2, P], F32, tag="hss")
                nc.scalar.activation(hs_s[:], hrow_s[:], AF.Copy, scale=0.01)
                nc.vector.tensor_tensor(out=hrow_s[:], in0=hrow_s[:], in1=hs_s[:],
                                        op=ALU.max)
                nc.sync.dma_start(heads_d[0:2, w * P:(w + 1) * P], hrow_p[:])
                nc.sync.dma_start(heads_d[2:4, w * P:(w + 1) * P], hrow_s[:])

            if "D" in PH:
                agg_pass(tbl2, close2)
            else:
                dbg2 = winp.tile([P, 64], BF16, tag="dbg2")
                nc.sync.dma_start(dbg2[:], tbl2[0:P, 0:64])
                nc.sync.dma_start(heads_d[0:4, 0:32].bitcast(BF16), dbg2[0:4, 0:64])

    nc0.compile()
    return nc0


def _split_pair(a):
    hi = a.astype(ml_dtypes.bfloat16)
    lo = (a - hi.astype(np.float32)).astype(ml_dtypes.bfloat16)
    return hi, lo


def make_in_maps(cfg, sched, idx_streams, dstrel, dinv, inputs):
    xt = np.zeros((cfg.IN, cfg.TBL), np.float32)
    xt[:, :cfg.N] = (np.asarray(inputs["x"], np.float32) * dinv[:, None]).T
    iota = np.broadcast_to(np.arange(P, dtype=np.float32), (P, P)).astype(
        ml_dtypes.bfloat16).copy()
    f32 = lambda k: np.asarray(inputs[k], np.float32)

    in_maps = []
    for c in range(cfg.NC):
        dv = np.zeros(cfg.WPAD, np.float32)
        dv[:cfg.BLK] = dinv[c * cfg.BLK:(c + 1) * cfg.BLK]
        dinvr = np.broadcast_to(dv, (cfg.H, cfg.WPAD)).copy()
        dinvc = dv.reshape(cfg.NW, P).T.copy()
        m = {
            "xt": xt, "dr": dstrel[c], "iota": iota,
            "dinvr": dinvr, "dinvc": dinvc,
            "w1": f32("W1"), "w2": f32("W2"),
            "wp1": f32("Wp1"), "wp2": f32("Wp2"),
            "wy": np.concatenate([f32("Wy1"), f32("Wy0")], axis=1),
            "b1": f32("b1").reshape(-1, 1),
            "b2": f32("b2").reshape(-1, 1),
            "bp1": f32("bp1").reshape(-1, 1),
            "bp2": f32("bp2").reshape(-1, 1),
            "by": np.concatenate([f32("by1"), f32("by0")]).reshape(-1, 1),
        }
        for b in range(cfg.NB):
            m[f"ix{b}"] = idx_streams[c][b]
        in_maps.append(m)
    return in_maps


def run(cfg, x, edge_index, W1, b1, W2, b2, Wy1, by1, Wy0, by0, Wp1, bp1, Wp2, bp2,
        trace=False):
    sched, idx_streams, dstrel, dinv = preprocess(cfg, x, edge_index)
    nc0 = build_program(cfg, sched)
    inputs = dict(x=x, W1=W1, b1=b1, W2=W2, b2=b2, Wy1=Wy1, by1=by1, Wy0=Wy0,
                  by0=by0, Wp1=Wp1, bp1=bp1, Wp2=Wp2, bp2=bp2)
    in_maps = make_in_maps(cfg, sched, idx_streams, dstrel, dinv, inputs)

    res = run_bass_kernel_spmd(nc0, in_maps, core_ids=list(range(cfg.NC)),
                               trace=trace)
    xz2 = np.concatenate(
        [res.results[c]["xz2t"][:, :cfg.BLK].T for c in range(cfg.NC)], axis=0)
    heads = np.concatenate(
        [res.results[c]["headst"][:, :cfg.BLK] for c in range(cfg.NC)], axis=1)
    tprob = heads[0:2, :].T.copy()
    s1 = heads[2, :]
    s0 = heads[3, :]
    return xz2, tprob, s1, s0, res


CFG = Cfg(n=100000, in_dim=128, h=64, ncores=8, nblocks=4)


def kernel(x, edge_index, treat_idx, control_idx,
           W1, b1, W2, b2, Wy1, by1, Wy0, by0, Wp1, bp1, Wp2, bp2):
    x = np.asarray(x)
    edge_index = np.asarray(edge_index)
    treat_idx = np.asarray(treat_idx)
    control_idx = np.asarray(control_idx)
    xz2, tprob, s1, s0, _ = run(
        CFG, x, edge_index, W1, b1, W2, b2, Wy1, by1, Wy0, by0, Wp1, bp1, Wp2, bp2)
    y1 = s1[treat_idx].astype(np.float32)
    yc0 = s0[treat_idx].astype(np.float32)
    y0 = s0[control_idx].astype(np.float32)
    yc1 = s1[control_idx].astype(np.float32)
    return (y1, yc0, y0, yc1, tprob.astype(np.float32), xz2.astype(np.float32))
